# revision 2
# baseline (speedup 1.0000x reference)
"""Trainium2 Bass kernel: hard-negative miner (masked top-5 indices over
50257 classes), data-parallel over 8 NeuronCores (1024 rows each).

Two-tensor strategy: the host uploads BOTH the f32 logits (padded) and an
fp16 copy. The device streams only the fp16 copy (~103 MB/core instead of
~207 MB) to find each row's top-8 candidate subchunks of 128 columns
(windowed max via a tensor_tensor fold tree, which runs in the DVE's 2x
16-bit packed perf mode; tensor_reduce is locked to 1x). It then gathers
just those subchunks from the f32 original (~4 MB/core, ascending column
order) and computes the exact masked top-5 on full-precision values, so
fp16 rounding only influences which subchunks are fetched, never the final
comparison. Candidate top-8-by-subchunk-max covers the top-6 values of a
row (at most 5 values exceed the 6th, plus tie slack); the 6th is needed
when the label is dropped. Verified exact (0/8192 mismatches) on the
reference dataset.
"""

import sys

sys.path.insert(0, "/opt/trn_rl_repo")

import numpy as np

import concourse.bass as bass
import concourse.mybir as mybir
from concourse import bacc, bass_utils
from concourse.tile import TileContext

B = 8192
N = 50257
TOP_K = 5
NCORES = 8
R = B // NCORES  # 1024
P = 128
T = R // P  # 8
NEG = -1.0e30

F32 = mybir.dt.float32
F16 = mybir.dt.float16
I32 = mybir.dt.int32
U32 = mybir.dt.uint32
AX = mybir.AxisListType.X
OP = mybir.AluOpType


def _splits(S, nblk):
    q, r = divmod(S, nblk)
    ws = [q + (1 if i < r else 0) for i in range(nblk)]
    ws.sort(key=lambda w: w % 2)  # odd widths last -> even c0 starts
    return ws


def build_bass(rep: int = 1, L: int = 128, nblk: int = 4,
               stop_after: str = "all", gather_lag: int = 1, blk_bufs: int = 2):
    S = (N + L - 1) // L
    blk_s = _splits(S, nblk)

    nc = bacc.Bacc("TRN2", num_devices=NCORES)
    x = nc.dram_tensor("x", (R, S * L), F32, kind="ExternalInput")
    xh = nc.dram_tensor("xh", (R, S * L), F16, kind="ExternalInput")
    labf = nc.dram_tensor("labf", (R, 1), F32, kind="ExternalInput")
    basec = nc.dram_tensor("basec", (P, T), F32, kind="ExternalInput")
    out = nc.dram_tensor("out", (R, TOP_K), I32, kind="ExternalOutput")

    def stream_tile(t, pb, pe):
        m_t = pe.tile([P, S], F16, tag="m")
        c0 = 0
        for bi, ws in enumerate(blk_s):
            blk = pb.tile([P, ws * L], F16, tag="blk")
            eng = [nc.sync, nc.scalar][bi % 2]
            eng.dma_start(
                out=blk[:, :],
                in_=xh[t * P : (t + 1) * P, c0 * L : (c0 + ws) * L],
            )
            # fold tree via tensor_tensor max (2x 16-bit DVE perf mode),
            # final 4->1 via tensor_reduce (tiny, 1x is fine)
            cur = blk
            w = L
            while w > 4:
                h = w // 2
                nxt = pb.tile([P, ws * h], F16, tag=f"fold{h}")
                v = cur[:].rearrange("p (s l) -> p s l", l=w)
                nc.vector.tensor_tensor(
                    out=nxt[:].rearrange("p (s l) -> p s l", l=h),
                    in0=v[:, :, 0:h],
                    in1=v[:, :, h:w],
                    op=OP.max,
                )
                cur = nxt
                w = h
            nc.vector.tensor_reduce(
                out=m_t[:, c0 : c0 + ws],
                in_=cur[:].rearrange("p (s l) -> p s l", l=w),
                axis=AX,
                op=OP.max,
            )
            c0 += ws
        return m_t

    def pick_tile(t, pe, m_t, base_t, psall):
        c8 = pe.tile([P, 8], F16, tag="c8")
        pidx = pe.tile([P, 8], U32, tag="pidx")
        nc.vector.max(out=c8[:, :], in_=m_t[:, :])
        nc.vector.max_index(out=pidx[:, :], in_max=c8[:, :], in_values=m_t[:, :])
        pa = pe.tile([P, 8], F32, tag="pa")
        pb2 = pe.tile([P, 8], F32, tag="pb2")
        nc.vector.tensor_copy(out=pa[:, :], in_=pidx[:, :])
        cur, nxt = pa, pb2
        for r in range(8):
            if r % 2 == 0:
                nc.vector.tensor_tensor(
                    out=nxt[:, 0::2], in0=cur[:, 0::2], in1=cur[:, 1::2], op=OP.min
                )
                nc.vector.tensor_tensor(
                    out=nxt[:, 1::2], in0=cur[:, 0::2], in1=cur[:, 1::2], op=OP.max
                )
            else:
                nc.vector.tensor_tensor(
                    out=nxt[:, 1:7:2], in0=cur[:, 1:7:2], in1=cur[:, 2:8:2], op=OP.min
                )
                nc.vector.tensor_tensor(
                    out=nxt[:, 2:8:2], in0=cur[:, 1:7:2], in1=cur[:, 2:8:2], op=OP.max
                )
                nc.vector.tensor_copy(out=nxt[:, 0::7], in_=cur[:, 0::7])
            cur, nxt = nxt, cur
        nc.vector.tensor_copy(out=psall[:, t * 8 : (t + 1) * 8], in_=cur[:, :])
        offs_f = pe.tile([P, 8], F32, tag="offs_f")
        offs_i = pe.tile([P, 8], I32, tag="offs_i")
        nc.vector.tensor_tensor(
            out=offs_f[:, :],
            in0=cur[:, :],
            in1=base_t[:, t : t + 1].to_broadcast([P, 8]),
            op=OP.add,
        )
        nc.vector.tensor_copy(out=offs_i[:, :], in_=offs_f[:, :])
        return offs_i

    def gather_tile(t, offs_i, gall):
        xflat = x.ap().rearrange("r (s l) -> (r s) l", l=L)
        for j in range(8):
            nc.gpsimd.indirect_dma_start(
                out=gall[:, t * 8 * L + j * L : t * 8 * L + (j + 1) * L],
                out_offset=None,
                in_=xflat,
                in_offset=bass.IndirectOffsetOnAxis(ap=offs_i[:, j : j + 1], axis=0),
            )

    def scans_tile(t, g8all, qall, gall):
        nc.vector.max(
            out=g8all[:, t * 8 : (t + 1) * 8],
            in_=gall[:, t * 8 * L : (t + 1) * 8 * L],
        )
        nc.vector.max_index(
            out=qall[:, t * 8 : (t + 1) * 8],
            in_max=g8all[:, t * 8 : (t + 1) * 8],
            in_values=gall[:, t * 8 * L : (t + 1) * 8 * L],
        )

    def final_tail(pe, lab8, psall, gall, g8all, qall, t0, nt):
        q = qall[:, t0 * 8 : (t0 + nt) * 8]

        def r3(ap):
            return ap[:].rearrange("p (t j) -> p t j", j=8)

        sid_u = pe.tile([P, nt * 8], U32, tag="sid_u")
        win_u = pe.tile([P, nt * 8], U32, tag="win_u")
        nc.vector.tensor_scalar(
            out=sid_u[:, :], in0=q, scalar1=int(L).bit_length() - 1,
            scalar2=None, op0=OP.logical_shift_right,
        )
        nc.vector.tensor_scalar(
            out=win_u[:, :], in0=q, scalar1=L - 1, scalar2=None,
            op0=OP.bitwise_and,
        )
        sid_f = pe.tile([P, nt * 8], F32, tag="sid_f")
        win_f = pe.tile([P, nt * 8], F32, tag="win_f")
        nc.vector.tensor_copy(out=sid_f[:, :], in_=sid_u[:, :])
        nc.vector.tensor_copy(out=win_f[:, :], in_=win_u[:, :])

        ps3 = psall[:].rearrange("p (t j) -> p t j", j=8)[:, t0 : t0 + nt, :]
        acc = pe.tile([P, nt * 8], F32, tag="acc")
        eqt = pe.tile([P, nt * 8], F32, tag="eqt")
        trm = pe.tile([P, nt * 8], F32, tag="trm")
        for c in range(8):
            nc.vector.tensor_scalar(
                out=eqt[:, :], in0=sid_f[:, :], scalar1=float(c), scalar2=None,
                op0=OP.is_equal,
            )
            pc = ps3[:, :, c : c + 1].to_broadcast([P, nt, 8])
            dst = acc if c == 0 else trm
            nc.vector.tensor_tensor(out=r3(dst), in0=r3(eqt), in1=pc, op=OP.mult)
            if c > 0:
                nc.vector.tensor_tensor(
                    out=acc[:, :], in0=acc[:, :], in1=trm[:, :], op=OP.add
                )
        gidx = pe.tile([P, nt * 8], F32, tag="gidx")
        nc.vector.tensor_scalar(
            out=acc[:, :], in0=acc[:, :], scalar1=float(L), scalar2=None, op0=OP.mult
        )
        nc.vector.tensor_tensor(
            out=gidx[:, :], in0=acc[:, :], in1=win_f[:, :], op=OP.add
        )

        valid = pe.tile([P, nt * 8], F32, tag="valid")
        lab3 = lab8[:].rearrange("p (t one) -> p t one", one=1)
        nc.vector.tensor_tensor(
            out=r3(valid),
            in0=r3(gidx),
            in1=lab3[:, t0 : t0 + nt, 0:1].to_broadcast([P, nt, 8]),
            op=OP.not_equal,
        )
        c1 = pe.tile([P, nt * 8], F32, tag="c1")
        c2 = pe.tile([P, nt * 8], F32, tag="c2")
        c4 = pe.tile([P, nt * 8], F32, tag="c4")
        v3, c13, c23, c43 = r3(valid), r3(c1), r3(c2), r3(c4)
        nc.vector.tensor_copy(out=c1[:, :], in_=valid[:, :])
        nc.vector.tensor_tensor(
            out=c13[:, :, 1:8], in0=v3[:, :, 1:8], in1=v3[:, :, 0:7], op=OP.add
        )
        nc.vector.tensor_copy(out=c2[:, :], in_=c1[:, :])
        nc.vector.tensor_tensor(
            out=c23[:, :, 2:8], in0=c13[:, :, 2:8], in1=c13[:, :, 0:6], op=OP.add
        )
        nc.vector.tensor_copy(out=c4[:, :], in_=c2[:, :])
        nc.vector.tensor_tensor(
            out=c43[:, :, 4:8], in0=c23[:, :, 4:8], in1=c23[:, :, 0:4], op=OP.add
        )

        out5f = pe.tile([P, nt * TOP_K], F32, tag="out5f")
        out5f3 = out5f[:].rearrange("p (t k) -> p t k", k=TOP_K)
        sel = pe.tile([P, nt * 8], F32, tag="sel")
        for k in range(TOP_K):
            nc.vector.tensor_scalar(
                out=eqt[:, :], in0=c4[:, :], scalar1=float(k + 1), scalar2=None,
                op0=OP.is_equal,
            )
            nc.vector.tensor_tensor(
                out=sel[:, :], in0=eqt[:, :], in1=valid[:, :], op=OP.mult
            )
            nc.vector.tensor_tensor(
                out=sel[:, :], in0=sel[:, :], in1=gidx[:, :], op=OP.mult
            )
            nc.vector.tensor_reduce(
                out=out5f3[:, :, k : k + 1], in_=r3(sel), axis=AX, op=OP.add
            )
        out5i = pe.tile([P, nt * TOP_K], I32, tag="out5i")
        nc.vector.tensor_copy(out=out5i[:, :], in_=out5f[:, :])
        nc.sync.dma_start(
            out=out.ap().rearrange("(t p) k -> p t k", p=P)[:, t0 : t0 + nt, :],
            in_=out5i[:].rearrange("p (t k) -> p t k", k=TOP_K),
        )

    with TileContext(nc) as tc:
        with (
            tc.tile_pool(name="persist", bufs=1) as pp,
            tc.tile_pool(name="blk", bufs=blk_bufs) as pb,
            tc.tile_pool(name="epi", bufs=2) as pe,
        ):
            lab8 = pp.tile([P, T], F32)
            base_t = pp.tile([P, T], F32)
            nc.sync.dma_start(
                out=lab8[:, :],
                in_=labf.ap().rearrange("(t p) one -> p (t one)", p=P),
            )
            nc.sync.dma_start(out=base_t[:, :], in_=basec[:, :])
            for _ in range(rep):
                psall = pp.tile([P, T * 8], F32, tag="psall")
                gall = pp.tile([P, T * 8 * L], F32, tag="gall")
                g8all = pp.tile([P, T * 8], F32, tag="g8all")
                qall = pp.tile([P, T * 8], U32, tag="qall")
                offs = [None] * T
                for t in range(T):
                    m_t = stream_tile(t, pb, pe)
                    if stop_after == "scan":
                        continue
                    offs[t] = pick_tile(t, pe, m_t, base_t, psall)
                    tg = t - gather_lag
                    if tg >= 0:
                        gather_tile(tg, offs[tg], gall)
                        scans_tile(tg, g8all, qall, gall)
                if stop_after == "scan":
                    continue
                for tg in range(max(0, T - gather_lag), T):
                    gather_tile(tg, offs[tg], gall)
                    scans_tile(tg, g8all, qall, gall)
                final_tail(pe, lab8, psall, gall, g8all, qall, 0, T)

    nc.compile()
    return nc


def make_in_maps(teacher_logits: np.ndarray, labels: np.ndarray, L: int = 128):
    S = (N + L - 1) // L
    W = S * L
    xpad = np.full((B, W), NEG, dtype=np.float32)
    xpad[:, :N] = teacher_logits
    xh = xpad.astype(np.float16)
    labf = labels.astype(np.float32).reshape(B, 1)
    p = np.arange(P, dtype=np.float32).reshape(P, 1)
    t = np.arange(T, dtype=np.float32).reshape(1, T)
    basec = ((t * P + p) * S).astype(np.float32)
    in_maps = []
    for c in range(NCORES):
        in_maps.append(
            {
                "x": xpad[c * R : (c + 1) * R],
                "xh": xh[c * R : (c + 1) * R],
                "labf": labf[c * R : (c + 1) * R],
                "basec": basec,
            }
        )
    return in_maps


def kernel(teacher_logits: np.ndarray, labels: np.ndarray, nc=None) -> np.ndarray:
    if nc is None:
        nc = _get_nc()
    in_maps = make_in_maps(np.asarray(teacher_logits), np.asarray(labels))
    res = bass_utils.run_bass_kernel_spmd(nc, in_maps, core_ids=list(range(NCORES)))
    out = np.concatenate([r["out"] for r in res.results], axis=0)
    return out.astype(np.int32)


_NC_CACHE = None


def _get_nc():
    global _NC_CACHE
    if _NC_CACHE is None:
        _NC_CACHE = build_bass()
    return _NC_CACHE


# revision 3
# speedup vs baseline: 5.1084x; 5.1084x over previous
"""Trainium2 Bass kernel: hard-negative miner (masked top-5 indices over
50257 classes), data-parallel over 8 NeuronCores (1024 rows each).

Two-tensor strategy: the host uploads BOTH the f32 logits (padded) and an
fp16 copy. The device streams only the fp16 copy (~103 MB/core instead of
~207 MB) to find each row's top-8 candidate subchunks of 128 columns
(windowed max via a tensor_tensor fold tree, which runs in the DVE's 2x
16-bit packed perf mode; tensor_reduce is locked to 1x). It then gathers
just those subchunks from the f32 original (~4 MB/core, ascending column
order) and computes the exact masked top-5 on full-precision values, so
fp16 rounding only influences which subchunks are fetched, never the final
comparison. Candidate top-8-by-subchunk-max covers the top-6 values of a
row (at most 5 values exceed the 6th, plus tie slack); the 6th is needed
when the label is dropped. Verified exact (0/8192 mismatches) on the
reference dataset.
"""

import sys

sys.path.insert(0, "/opt/trn_rl_repo")

import numpy as np

import concourse.bass as bass
import concourse.mybir as mybir
from concourse import bacc, bass_utils
from concourse.tile import TileContext

B = 8192
N = 50257
TOP_K = 5
NCORES = 8
R = B // NCORES  # 1024
P = 128
T = R // P  # 8
NEG = -1.0e30

F32 = mybir.dt.float32
F16 = mybir.dt.float16
I32 = mybir.dt.int32
U32 = mybir.dt.uint32
AX = mybir.AxisListType.X
OP = mybir.AluOpType


def _splits(S, nblk):
    q, r = divmod(S, nblk)
    ws = [q + (1 if i < r else 0) for i in range(nblk)]
    ws.sort(key=lambda w: w % 2)  # odd widths last -> even c0 starts
    return ws


def build_bass(rep: int = 1, L: int = 128, nblk: int = 4,
               stop_after: str = "all", gather_lag: int = 1, blk_bufs: int = 2):
    S = (N + L - 1) // L
    blk_s = _splits(S, nblk)

    nc = bacc.Bacc("TRN2", num_devices=NCORES)
    x = nc.dram_tensor("x", (R, S * L), F32, kind="ExternalInput")
    xh = nc.dram_tensor("xh", (R, S * L), F16, kind="ExternalInput")
    labf = nc.dram_tensor("labf", (R, 1), F32, kind="ExternalInput")
    basec = nc.dram_tensor("basec", (P, T), F32, kind="ExternalInput")
    out = nc.dram_tensor("out", (R, TOP_K), I32, kind="ExternalOutput")

    def stream_tile(t, pb, pe):
        m_t = pe.tile([P, S], F16, tag="m")
        c0 = 0
        for bi, ws in enumerate(blk_s):
            blk = pb.tile([P, ws * L], F16, tag="blk")
            eng = [nc.sync, nc.scalar][bi % 2]
            eng.dma_start(
                out=blk[:, :],
                in_=xh[t * P : (t + 1) * P, c0 * L : (c0 + ws) * L],
            )
            # fold tree via tensor_tensor max (2x 16-bit DVE perf mode),
            # final 4->1 via tensor_reduce (tiny, 1x is fine)
            cur = blk
            w = L
            while w > 4:
                h = w // 2
                nxt = pb.tile([P, ws * h], F16, tag=f"fold{h}")
                v = cur[:].rearrange("p (s l) -> p s l", l=w)
                nc.vector.tensor_tensor(
                    out=nxt[:].rearrange("p (s l) -> p s l", l=h),
                    in0=v[:, :, 0:h],
                    in1=v[:, :, h:w],
                    op=OP.max,
                )
                cur = nxt
                w = h
            nc.vector.tensor_reduce(
                out=m_t[:, c0 : c0 + ws],
                in_=cur[:].rearrange("p (s l) -> p s l", l=w),
                axis=AX,
                op=OP.max,
            )
            c0 += ws
        return m_t

    def pick_small(t, pe, m_t, pidxf):
        c8 = pe.tile([P, 8], F16, tag="c8")
        pidx = pe.tile([P, 8], U32, tag="pidx")
        nc.vector.max(out=c8[:, :], in_=m_t[:, :])
        nc.vector.max_index(out=pidx[:, :], in_max=c8[:, :], in_values=m_t[:, :])
        nc.vector.tensor_copy(out=pidxf[:, t * 8 : (t + 1) * 8], in_=pidx[:, :])

    def group_pick(g, pe, pidxf, base_t, psall, nt=4):
        t0 = g * nt
        pa = pe.tile([P, nt * 8], F32, tag="pa")
        pb2 = pe.tile([P, nt * 8], F32, tag="pb2")
        nc.vector.tensor_copy(out=pa[:, :], in_=pidxf[:, t0 * 8 : (t0 + nt) * 8])

        def r3(ap):
            return ap[:].rearrange("p (t j) -> p t j", j=8)

        cur, nxt = pa, pb2
        for r in range(8):
            c3, n3 = r3(cur), r3(nxt)
            if r % 2 == 0:
                nc.vector.tensor_tensor(
                    out=n3[:, :, 0::2], in0=c3[:, :, 0::2], in1=c3[:, :, 1::2],
                    op=OP.min,
                )
                nc.vector.tensor_tensor(
                    out=n3[:, :, 1::2], in0=c3[:, :, 0::2], in1=c3[:, :, 1::2],
                    op=OP.max,
                )
            else:
                nc.vector.tensor_tensor(
                    out=n3[:, :, 1:7:2], in0=c3[:, :, 1:7:2], in1=c3[:, :, 2:8:2],
                    op=OP.min,
                )
                nc.vector.tensor_tensor(
                    out=n3[:, :, 2:8:2], in0=c3[:, :, 1:7:2], in1=c3[:, :, 2:8:2],
                    op=OP.max,
                )
                nc.vector.tensor_copy(out=n3[:, :, 0::7], in_=c3[:, :, 0::7])
            cur, nxt = nxt, cur
        nc.vector.tensor_copy(out=psall[:, t0 * 8 : (t0 + nt) * 8], in_=cur[:, :])
        offs_f = pe.tile([P, nt * 8], F32, tag="offs_f")
        offs_i = pe.tile([P, nt * 8], I32, tag="offs_i")
        nc.vector.tensor_tensor(
            out=r3(offs_f),
            in0=r3(cur),
            in1=base_t[:].rearrange("p (t one) -> p t one", one=1)[
                :, t0 : t0 + nt, 0:1
            ].to_broadcast([P, nt, 8]),
            op=OP.add,
        )
        nc.vector.tensor_copy(out=offs_i[:, :], in_=offs_f[:, :])
        return offs_i

    def gather_tile(t, offs_i, c0, gall):
        xflat = x.ap().rearrange("r (s l) -> (r s) l", l=L)
        for j in range(8):
            nc.gpsimd.indirect_dma_start(
                out=gall[:, t * 8 * L + j * L : t * 8 * L + (j + 1) * L],
                out_offset=None,
                in_=xflat,
                in_offset=bass.IndirectOffsetOnAxis(
                    ap=offs_i[:, c0 + j : c0 + j + 1], axis=0
                ),
            )

    def scans_tile(t, g8all, qall, gall):
        nc.vector.max(
            out=g8all[:, t * 8 : (t + 1) * 8],
            in_=gall[:, t * 8 * L : (t + 1) * 8 * L],
        )
        nc.vector.max_index(
            out=qall[:, t * 8 : (t + 1) * 8],
            in_max=g8all[:, t * 8 : (t + 1) * 8],
            in_values=gall[:, t * 8 * L : (t + 1) * 8 * L],
        )

    def final_tail(pe, lab8, psall, gall, g8all, qall, t0, nt):
        q = qall[:, t0 * 8 : (t0 + nt) * 8]

        def r3(ap):
            return ap[:].rearrange("p (t j) -> p t j", j=8)

        sid_u = pe.tile([P, nt * 8], U32, tag="sid_u")
        win_u = pe.tile([P, nt * 8], U32, tag="win_u")
        nc.vector.tensor_scalar(
            out=sid_u[:, :], in0=q, scalar1=int(L).bit_length() - 1,
            scalar2=None, op0=OP.logical_shift_right,
        )
        nc.vector.tensor_scalar(
            out=win_u[:, :], in0=q, scalar1=L - 1, scalar2=None,
            op0=OP.bitwise_and,
        )
        sid_f = pe.tile([P, nt * 8], F32, tag="sid_f")
        win_f = pe.tile([P, nt * 8], F32, tag="win_f")
        nc.vector.tensor_copy(out=sid_f[:, :], in_=sid_u[:, :])
        nc.vector.tensor_copy(out=win_f[:, :], in_=win_u[:, :])

        ps3 = psall[:].rearrange("p (t j) -> p t j", j=8)[:, t0 : t0 + nt, :]
        acc = pe.tile([P, nt * 8], F32, tag="acc")
        eqt = pe.tile([P, nt * 8], F32, tag="eqt")
        trm = pe.tile([P, nt * 8], F32, tag="trm")
        for c in range(8):
            nc.vector.tensor_scalar(
                out=eqt[:, :], in0=sid_f[:, :], scalar1=float(c), scalar2=None,
                op0=OP.is_equal,
            )
            pc = ps3[:, :, c : c + 1].to_broadcast([P, nt, 8])
            dst = acc if c == 0 else trm
            nc.vector.tensor_tensor(out=r3(dst), in0=r3(eqt), in1=pc, op=OP.mult)
            if c > 0:
                nc.vector.tensor_tensor(
                    out=acc[:, :], in0=acc[:, :], in1=trm[:, :], op=OP.add
                )
        gidx = pe.tile([P, nt * 8], F32, tag="gidx")
        nc.vector.tensor_scalar(
            out=acc[:, :], in0=acc[:, :], scalar1=float(L), scalar2=None, op0=OP.mult
        )
        nc.vector.tensor_tensor(
            out=gidx[:, :], in0=acc[:, :], in1=win_f[:, :], op=OP.add
        )

        valid = pe.tile([P, nt * 8], F32, tag="valid")
        lab3 = lab8[:].rearrange("p (t one) -> p t one", one=1)
        nc.vector.tensor_tensor(
            out=r3(valid),
            in0=r3(gidx),
            in1=lab3[:, t0 : t0 + nt, 0:1].to_broadcast([P, nt, 8]),
            op=OP.not_equal,
        )
        c1 = pe.tile([P, nt * 8], F32, tag="c1")
        c2 = pe.tile([P, nt * 8], F32, tag="c2")
        c4 = pe.tile([P, nt * 8], F32, tag="c4")
        v3, c13, c23, c43 = r3(valid), r3(c1), r3(c2), r3(c4)
        nc.vector.tensor_copy(out=c1[:, :], in_=valid[:, :])
        nc.vector.tensor_tensor(
            out=c13[:, :, 1:8], in0=v3[:, :, 1:8], in1=v3[:, :, 0:7], op=OP.add
        )
        nc.vector.tensor_copy(out=c2[:, :], in_=c1[:, :])
        nc.vector.tensor_tensor(
            out=c23[:, :, 2:8], in0=c13[:, :, 2:8], in1=c13[:, :, 0:6], op=OP.add
        )
        nc.vector.tensor_copy(out=c4[:, :], in_=c2[:, :])
        nc.vector.tensor_tensor(
            out=c43[:, :, 4:8], in0=c23[:, :, 4:8], in1=c23[:, :, 0:4], op=OP.add
        )

        out5f = pe.tile([P, nt * TOP_K], F32, tag="out5f")
        out5f3 = out5f[:].rearrange("p (t k) -> p t k", k=TOP_K)
        sel = pe.tile([P, nt * 8], F32, tag="sel")
        for k in range(TOP_K):
            nc.vector.tensor_scalar(
                out=eqt[:, :], in0=c4[:, :], scalar1=float(k + 1), scalar2=None,
                op0=OP.is_equal,
            )
            nc.vector.tensor_tensor(
                out=sel[:, :], in0=eqt[:, :], in1=valid[:, :], op=OP.mult
            )
            nc.vector.tensor_tensor(
                out=sel[:, :], in0=sel[:, :], in1=gidx[:, :], op=OP.mult
            )
            nc.vector.tensor_reduce(
                out=out5f3[:, :, k : k + 1], in_=r3(sel), axis=AX, op=OP.add
            )
        out5i = pe.tile([P, nt * TOP_K], I32, tag="out5i")
        nc.vector.tensor_copy(out=out5i[:, :], in_=out5f[:, :])
        nc.sync.dma_start(
            out=out.ap().rearrange("(t p) k -> p t k", p=P)[:, t0 : t0 + nt, :],
            in_=out5i[:].rearrange("p (t k) -> p t k", k=TOP_K),
        )

    with TileContext(nc) as tc:
        with (
            tc.tile_pool(name="persist", bufs=1) as pp,
            tc.tile_pool(name="blk", bufs=blk_bufs) as pb,
            tc.tile_pool(name="epi", bufs=2) as pe,
        ):
            lab8 = pp.tile([P, T], F32)
            base_t = pp.tile([P, T], F32)
            nc.sync.dma_start(
                out=lab8[:, :],
                in_=labf.ap().rearrange("(t p) one -> p (t one)", p=P),
            )
            nc.sync.dma_start(out=base_t[:, :], in_=basec[:, :])
            for _ in range(rep):
                psall = pp.tile([P, T * 8], F32, tag="psall")
                gall = pp.tile([P, T * 8 * L], F32, tag="gall")
                g8all = pp.tile([P, T * 8], F32, tag="g8all")
                qall = pp.tile([P, T * 8], U32, tag="qall")
                pidxf = pp.tile([P, T * 8], F32, tag="pidxf")
                for t in range(T):
                    m_t = stream_tile(t, pb, pe)
                    if stop_after == "scan":
                        continue
                    pick_small(t, pe, m_t, pidxf)
                    if t == 3:
                        og = group_pick(0, pe, pidxf, base_t, psall)
                        for tt in range(0, 4):
                            gather_tile(tt, og, tt * 8, gall)
                            scans_tile(tt, g8all, qall, gall)
                if stop_after == "scan":
                    continue
                og = group_pick(1, pe, pidxf, base_t, psall)
                for tt in range(4, 8):
                    gather_tile(tt, og, (tt - 4) * 8, gall)
                    scans_tile(tt, g8all, qall, gall)
                final_tail(pe, lab8, psall, gall, g8all, qall, 0, T)

    nc.compile()
    return nc


def make_in_maps(teacher_logits: np.ndarray, labels: np.ndarray, L: int = 128):
    S = (N + L - 1) // L
    W = S * L
    xpad = np.full((B, W), NEG, dtype=np.float32)
    xpad[:, :N] = teacher_logits
    xh = xpad.astype(np.float16)
    labf = labels.astype(np.float32).reshape(B, 1)
    p = np.arange(P, dtype=np.float32).reshape(P, 1)
    t = np.arange(T, dtype=np.float32).reshape(1, T)
    basec = ((t * P + p) * S).astype(np.float32)
    in_maps = []
    for c in range(NCORES):
        in_maps.append(
            {
                "x": xpad[c * R : (c + 1) * R],
                "xh": xh[c * R : (c + 1) * R],
                "labf": labf[c * R : (c + 1) * R],
                "basec": basec,
            }
        )
    return in_maps


def kernel(teacher_logits: np.ndarray, labels: np.ndarray, nc=None) -> np.ndarray:
    if nc is None:
        nc = _get_nc()
    in_maps = make_in_maps(np.asarray(teacher_logits), np.asarray(labels))
    res = bass_utils.run_bass_kernel_spmd(nc, in_maps, core_ids=list(range(NCORES)))
    out = np.concatenate([r["out"] for r in res.results], axis=0)
    return out.astype(np.int32)


_NC_CACHE = None


def _get_nc():
    global _NC_CACHE
    if _NC_CACHE is None:
        _NC_CACHE = build_bass()
    return _NC_CACHE
